# revision 10
# baseline (speedup 1.0000x reference)
"""AdditiveAttention on 8 TRN2 NeuronCores — data-parallel over batch.

Algebraic restructuring: instead of materializing the [Lq,Lk,H] tanh
intermediate (33.5M elementwise ops/core), approximate

    tanh(z) ~= clin*z + alpha*sin(w*z)

(coefficients fit at runtime to the data's projection ranges; end-to-end
rel-err ~6e-3 vs the 2e-2 gate) and expand via the angle-sum identity

    sin(w(a+b)) = sin(wa)*(1-2*sin^2(wb/2)) + (1-2*sin^2(wa/2))*sin(wb)

so scores[q,k] = sum_h wv_h*tanh(qh+kh) collapse to a 3-row-per-h-chunk
matmul contraction (q-only terms drop out of the softmax):

    row 0: [wv*clin]_const(q)        x  kh_raw(k)        (linear term)
    row 1: [-2*a*wv*sin(w*qh)]       x  sin^2(w*kh/2)
    row 2: [a*wv*(1-2sin^2(w*qh/2))] x  sin(w*kh)

Features are sines of the small [Lq,H]/[Lk,H] projections; cos comes from
the half-angle square (respects Sin's [-pi,pi] hw range).  ACT evaluates
only Sin + final Exp (table load hidden behind a dummy exp); squares and
folds run on DVE; the kh->bf16 copy runs on idle GPSIMD.  Only
ceil(max_vlen/128) key slabs are processed; masking follows the
zeroed-values + mask-column trick (vlen==0 -> wv=0 -> uniform).
DMAs are bundled (one HWDGE generation each) and ordered by need since
the cost model serializes all DMA transfers on one resource.
"""

import ml_dtypes
import numpy as np

B, LQ, LK, D, H, DV = 8, 128, 1024, 512, 256, 512
NCORES = 8
HC = H // 128   # 2 h chunks
DC = D // 128   # 4 contraction chunks
NROW = 3        # contraction rows per h-chunk

# runtime-fit parameters (overwritten by _make_in_maps; affect numerics
# only, never the schedule)
_CFG = {"w": 1.30, "alph": 0.44, "clin": 0.35, "kce": 8}


def _build_program():
    import concourse.mybir as mybir
    import concourse.tile as tile
    from concourse import bacc

    f32 = mybir.dt.float32
    bf16 = mybir.dt.bfloat16
    AF = mybir.ActivationFunctionType
    mult = mybir.AluOpType.mult
    add = mybir.AluOpType.add
    w = _CFG["w"]
    KCe = _CFG["kce"]
    LKe = KCe * 128
    KW1 = LKe - 512
    NG2 = KCe - 4
    NCC = 8 + (KCe + 1) // 2  # f32 cols: wvm2a|wva|wkv|mcol(bf16-packed)

    nc = bacc.Bacc(
        "TRN2",
        target_bir_lowering=False,
        debug=False,
        num_devices=NCORES,
    )

    # bundled inputs: one HWDGE generation per DMA, ordered by need
    wkt0_ext = nc.dram_tensor("wkt0", [D, H + 256], bf16, kind="ExternalInput").ap()
    wqt_ext = nc.dram_tensor("wqt", [D, H + LQ], bf16, kind="ExternalInput").ap()
    ktq1_ext = nc.dram_tensor("ktq1", [D, 256], bf16, kind="ExternalInput").ap()
    kt1_ext = nc.dram_tensor("kt1", [D, KW1], bf16, kind="ExternalInput").ap()
    consts_ext = nc.dram_tensor("consts", [128, NCC], f32, kind="ExternalInput").ap()
    val_ext = nc.dram_tensor("values", [LKe, DV], bf16, kind="ExternalInput").ap()
    out_ext = nc.dram_tensor("out", [LQ, DV], bf16, kind="ExternalOutput").ap()

    with tile.TileContext(nc) as tc:
        with (
            tc.tile_pool(name="const", bufs=1) as const,
            tc.tile_pool(name="pq", bufs=1, space="PSUM") as pq,
            tc.tile_pool(name="pk", bufs=1, space="PSUM") as pk,
            tc.tile_pool(name="psc", bufs=1, space="PSUM") as psc,
            tc.tile_pool(name="pout", bufs=1, space="PSUM") as pout,
        ):
            # ---- SBUF residents ----------------------------------------
            wkt0 = const.tile([128, DC, H + 256], bf16, tag="wkt0")
            wqt = const.tile([128, DC, H + LQ], bf16, tag="wqt")
            ktq1 = const.tile([128, DC, 256], bf16, tag="ktq1")
            kt1 = const.tile([128, DC, KW1], bf16, tag="kt1")
            consts = const.tile([128, NCC], f32, tag="consts")
            vals = const.tile([128, KCe, DV], bf16, tag="vals")
            ones = const.tile([128, LQ], bf16, tag="ones")
            wkvq = const.tile([128, DC, LQ], bf16, tag="wkvq")
            asin = const.tile([128, HC, LQ], bf16, tag="asin")
            ahalf = const.tile([128, HC, LQ], bf16, tag="ahalf")
            ata = const.tile([128, HC, LQ], bf16, tag="ata")
            Arows = const.tile([128, HC, 2, LQ], bf16, tag="Arows")
            # per-quarter/half feature tiles (dep tracking is tile-granular)
            tbA = const.tile([128, HC, 256], bf16, tag="tbA")
            sbA = const.tile([128, HC, 256], bf16, tag="sbA")
            bhA = const.tile([128, HC, 256], bf16, tag="bhA")
            tbB = const.tile([128, HC, 256], bf16, tag="tbB")
            sbB = const.tile([128, HC, 256], bf16, tag="sbB")
            bhB = const.tile([128, HC, 256], bf16, tag="bhB")
            tb1t = const.tile([128, HC, KW1], bf16, tag="tb1t")
            sb1t = const.tile([128, HC, KW1], bf16, tag="sb1t")
            bhalf1 = const.tile([128, HC, KW1], bf16, tag="bhalf1")
            pT1 = const.tile([128, 4, LQ], bf16, tag="pT1")
            pT2 = const.tile([128, NG2, LQ], bf16, tag="pT2")
            rinv = const.tile([LQ, 1], f32, tag="rinv")
            out_a = const.tile([LQ, DV // 2], bf16, tag="outa")
            out_b = const.tile([LQ, DV // 2], bf16, tag="outb")

            wk_sb = wkt0[:, :, 0:H]
            ksA = wkt0[:, :, H:H + 256]      # kT columns 0:256
            wq_sb = wqt[:, :, 0:H]
            qsT = wqt[:, :, H:H + LQ]
            wvm2a = consts[:, 0:HC]
            wva = consts[:, HC:2 * HC]
            wkv = consts[:, 4:4 + DC]
            mcol = consts[:, 8:NCC].bitcast(bf16)

            nc.vector.memset(ones[:], 1.0)

            # ---- DMAs (transfers serialize globally in the cost model) -
            nc.sync.dma_start(
                wkt0[:], wkt0_ext.rearrange("(c p) x -> p c x", p=128)
            )
            nc.sync.dma_start(
                wqt[:], wqt_ext.rearrange("(c p) x -> p c x", p=128)
            )
            nc.sync.dma_start(
                ktq1[:], ktq1_ext.rearrange("(c p) x -> p c x", p=128)
            )
            nc.sync.dma_start(
                kt1[:], kt1_ext.rearrange("(c p) x -> p c x", p=128)
            )
            nc.sync.dma_start(consts[:], consts_ext[:])
            # values gated behind kt1's landing so it never delays the k-side
            nc.gpsimd.tensor_copy(vals[0:1, 0, 0:1], kt1[0:1, 0, 0:1])
            nc.gpsimd.dma_start(
                vals[:], val_ext.rearrange("(c p) v -> p c v", p=128)
            )

            # ---- PSUM tiles (8 banks exactly) --------------------------
            qh = pq.tile([128, HC, LQ], f32, tag="qh")
            khA = pk.tile([128, HC, 256], f32, tag="khA")
            khB = pk.tile([128, HC, 256], f32, tag="khB")
            kh1 = pk.tile([128, HC, KW1], f32, tag="kh1", padded_shape=[128, HC, 512])
            scg1 = psc.tile([128, 4, LQ], f32, tag="scg1")
            scg2 = psc.tile([128, NG2, LQ], f32, tag="scg2", padded_shape=[128, 4, LQ])
            po = pout.tile([LQ, DV], f32, tag="po")

            # ---- PE spins: hold the clock through every dep wait -------
            def spins(n, tgt):
                for _ in range(n):
                    nc.tensor.matmul(
                        tgt, lhsT=ones[:, 0:128], rhs=ones[:, 0:LQ],
                        start=True, stop=True, skip_group_check=True,
                    )

            def proj(dst, wsb, src, kw):
                for hc in range(HC):
                    for dc in range(DC):
                        nc.tensor.matmul(
                            dst[:, hc, 0:kw],
                            lhsT=wsb[:, dc, hc * 128:(hc + 1) * 128],
                            rhs=src[:, dc, 0:kw],
                            start=(dc == 0),
                            stop=(dc == DC - 1),
                        )

            # PE queue: spins / projA / qh / projB / proj1 / scores
            spins(30, scg1[:, 0, :])
            proj(khA, wk_sb, ksA, 256)
            spins(2, scg1[:, 0, :])
            proj(qh, wq_sb, qsT, LQ)
            spins(5, scg1[:, 0, :])
            proj(khB, wk_sb, ktq1, 256)
            spins(4, scg1[:, 0, :])
            proj(kh1, wk_sb, kt1, KW1)

            # ---- ACT stream (Sin only until the final Exp) -------------
            nc.scalar.activation(bhA[:], khA[:, :, :], AF.Sin, scale=w / 2)
            nc.scalar.activation(asin[:, :, :], qh[:, :, :], AF.Sin, scale=w)
            nc.scalar.activation(ahalf[:, :, :], qh[:, :, :], AF.Sin, scale=w / 2)
            nc.scalar.activation(sbA[:], khA[:, :, :], AF.Sin, scale=w)
            nc.scalar.activation(bhB[:], khB[:, :, :], AF.Sin, scale=w / 2)
            nc.scalar.activation(sbB[:], khB[:, :, :], AF.Sin, scale=w)
            nc.scalar.activation(bhalf1[:], kh1[:, :, 0:KW1], AF.Sin, scale=w / 2)
            nc.scalar.activation(sb1t[:], kh1[:, :, 0:KW1], AF.Sin, scale=w)

            # ---- DVE: wkv broadcast, squares, coefficient folds --------
            for dc in range(DC):
                nc.vector.tensor_scalar(
                    wkvq[:, dc, :], ones[:, 0:LQ], wkv[:, dc:dc + 1],
                    None, mult,
                )
            nc.vector.tensor_mul(tbA[:], bhA[:], bhA[:])
            nc.vector.tensor_mul(ata[:], ahalf[:], ahalf[:])
            for hc in range(HC):
                nc.vector.tensor_scalar(
                    Arows[:, hc, 0, :], asin[:, hc, :],
                    wvm2a[:, hc:hc + 1], None, mult,
                )
                nc.vector.tensor_scalar(
                    Arows[:, hc, 1, :], ata[:, hc, :],
                    wvm2a[:, hc:hc + 1], wva[:, hc:hc + 1], mult, add,
                )
            nc.vector.tensor_mul(tbB[:], bhB[:], bhB[:])
            nc.vector.tensor_mul(tb1t[:], bhalf1[:], bhalf1[:])

            # ---- scores: 8 accumulating matmuls per key slab -----------
            # linear row straight from the kT SBUF tiles (contraction d),
            # then the two sine rows (contraction h)
            def score_slabs(s0, s1, sc, g0, ktile, tbt, sbt):
                for s in range(s0, s1):
                    lf = s - s0   # index into feature/kT tiles
                    lg = s - g0   # index into the score-group tile
                    for dc in range(DC):
                        nc.tensor.matmul(
                            sc[:, lg, :],
                            lhsT=ktile[:, dc, lf * 128:(lf + 1) * 128],
                            rhs=wkvq[:, dc, :],
                            start=(dc == 0),
                            stop=False,
                        )
                    for r, rows in enumerate((tbt, sbt)):
                        for hc in range(HC):
                            nc.tensor.matmul(
                                sc[:, lg, :],
                                lhsT=rows[:, hc, lf * 128:(lf + 1) * 128],
                                rhs=Arows[:, hc, r, :],
                                start=False,
                                stop=(r == 1 and hc == HC - 1),
                            )

            score_slabs(0, 2, scg1, 0, ksA, tbA, sbA)
            spins(4, po[:, 0:LQ])
            score_slabs(2, 4, scg1, 0, ktq1, tbB, sbB)
            spins(20, po[:, 0:LQ])
            score_slabs(4, KCe, scg2, 4, kt1, tb1t, sb1t)
            spins(16, po[:, 0:LQ])

            # ---- softmax exp (first exp carries the table load) --------
            nc.scalar.activation(pT1[:], scg1[:], AF.Exp)
            nc.scalar.activation(pT2[:], scg2[:, 0:NG2, :], AF.Exp)

            ssum = pq.tile([LQ, 1], f32, tag="qh", name="ssum")
            for s in range(KCe):
                pt = pT1[:, s, :] if s < 4 else pT2[:, s - 4, :]
                nc.tensor.matmul(
                    ssum[:, :], lhsT=pt, rhs=mcol[:, s:s + 1],
                    start=(s == 0), stop=(s == KCe - 1),
                    skip_group_check=True,
                )
            nc.vector.reciprocal(rinv[:], ssum[:])
            # attn @ values in column halves; each half is normalized and
            # stored as soon as its accumulation completes
            for ch, outt in ((0, out_a), (1, out_b)):
                c0 = ch * (DV // 2)
                for s in range(KCe):
                    pt = pT1[:, s, :] if s < 4 else pT2[:, s - 4, :]
                    nc.tensor.matmul(
                        po[:, c0:c0 + DV // 2], lhsT=pt,
                        rhs=vals[:, s, c0:c0 + DV // 2],
                        start=(s == 0), stop=(s == KCe - 1),
                        skip_group_check=True,
                    )
                nc.vector.tensor_scalar_mul(
                    outt[:], po[:, c0:c0 + DV // 2], rinv[:]
                )
                nc.sync.dma_start(out_ext[:, c0:c0 + DV // 2], outt[:])

    nc.compile()
    return nc


def _fit_tanh(qh, kh):
    """Fit tanh(z) ~= clin*z + a*sin(w*z); w capped so every Sin argument
    (incl. half-angles) stays within [-pi, pi] on both sides."""
    amax = float(np.abs(qh).max())
    bmax = float(np.abs(kh).max())
    cmax = max(amax, bmax, 1e-3)
    sig = float(np.sqrt(qh.var() + kh.var()))
    sig = sig if sig > 1e-6 else 1.0
    wcap = np.pi / cmax / 1.01
    zmax = (amax + bmax) * 1.03
    zg = np.linspace(-zmax, zmax, 2001)
    wgt = np.exp(-0.5 * (zg / sig) ** 2) + 1e-3
    tz = np.tanh(zg)
    sww = np.sqrt(wgt)
    best = None
    for f1 in np.linspace(0.80, 0.995, 14):
        ws = wcap * f1
        A = np.stack([zg, np.sin(ws * zg)], axis=1)
        Aw = A * sww[:, None]
        G = Aw.T @ Aw + 1e-6 * np.eye(2)
        coef = np.linalg.solve(G, Aw.T @ (tz * sww))
        if np.abs(coef).sum() > 20:
            continue
        err = A @ coef - tz
        rms = float(np.sqrt((err ** 2 * wgt).sum() / wgt.sum()))
        mx = float(np.abs(err).max())
        s = rms + 0.01 * mx
        if best is None or s < best[0]:
            best = (s, ws, coef)
    _, ws, coef = best
    return float(ws), float(coef[1]), float(coef[0])


def _make_in_maps(queries, keys, values, Wq, Wk, wv, valid_lens):
    bfr = lambda x: np.asarray(x, np.float32).astype(ml_dtypes.bfloat16).astype(np.float32)
    queries = np.asarray(queries, dtype=np.float32)
    keys = np.asarray(keys, dtype=np.float32)
    values = np.asarray(values, dtype=np.float32)
    Wq = np.ascontiguousarray(np.asarray(Wq, dtype=np.float32))
    Wk = np.ascontiguousarray(np.asarray(Wk, dtype=np.float32))
    wv = np.asarray(wv, dtype=np.float32)
    vlens = np.asarray(valid_lens)

    qh = bfr(queries).reshape(-1, D) @ bfr(Wq)
    kh = bfr(keys).reshape(-1, D) @ bfr(Wk)
    w, alph, clin = _fit_tanh(qh, kh)
    _CFG["w"], _CFG["alph"], _CFG["clin"] = w, alph, clin
    if np.any(vlens == 0):
        KCe = 8
    else:
        KCe = max(1, int(-(-int(vlens.max()) // 128)))
    _CFG["kce"] = KCe
    LKe = KCe * 128

    Wq_bf = Wq.astype(ml_dtypes.bfloat16)
    Wk_bf = Wk.astype(ml_dtypes.bfloat16)
    wvT = np.ascontiguousarray(wv.reshape(HC, 128).T)  # [p, hc], h = hc*128+p
    karange = np.arange(LKe).reshape(KCe, 128).T  # [p, kc] -> k index
    in_maps = []
    for c in range(NCORES):
        vlen = int(vlens[c])
        if vlen == 0:
            mcol = np.ones((128, KCe), dtype=np.float32)
            wv_c = np.zeros_like(wvT)
            vals_c = values[c, :LKe]
        else:
            mcol = (karange < vlen).astype(np.float32)
            wv_c = wvT
            vals_c = np.where(
                (np.arange(LKe) < vlen)[:, None], values[c, :LKe], 0.0
            )
        mcol_bf = mcol.astype(ml_dtypes.bfloat16)
        if KCe % 2:
            mcol_bf = np.concatenate(
                [mcol_bf, np.zeros((128, 1), ml_dtypes.bfloat16)], axis=1
            )
        mcol_f32 = np.ascontiguousarray(mcol_bf).view(np.float32)
        wv_full = wv_c.T.reshape(-1)                       # [H], h = hc*128+p
        wkv = (clin * (Wk @ wv_full)).reshape(DC, 128).T    # [p, dc]
        consts = np.concatenate(
            [-2.0 * alph * wv_c, alph * wv_c, wkv, mcol_f32], axis=1
        ).astype(np.float32)
        kT = np.ascontiguousarray(keys[c].T).astype(ml_dtypes.bfloat16)
        in_maps.append(
            {
                "wkt0": np.ascontiguousarray(
                    np.concatenate([Wk_bf, kT[:, 0:256]], axis=1)
                ),
                "wqt": np.ascontiguousarray(
                    np.concatenate(
                        [Wq_bf, queries[c].T.astype(ml_dtypes.bfloat16)], axis=1
                    )
                ),
                "ktq1": np.ascontiguousarray(kT[:, 256:512]),
                "kt1": np.ascontiguousarray(kT[:, 512:LKe]),
                "consts": np.ascontiguousarray(consts),
                "values": np.ascontiguousarray(vals_c).astype(ml_dtypes.bfloat16),
            }
        )
    return in_maps


def kernel(queries, keys, values, Wq, Wk, wv, valid_lens):
    from concourse.bass_utils import run_bass_kernel_spmd

    in_maps = _make_in_maps(queries, keys, values, Wq, Wk, wv, valid_lens)
    nc = _build_program()
    res = run_bass_kernel_spmd(nc, in_maps, core_ids=list(range(NCORES)))
    out = np.stack(
        [res.results[c]["out"].astype(np.float32) for c in range(NCORES)], axis=0
    )
    return out


# revision 11
# speedup vs baseline: 1.0111x; 1.0111x over previous
"""AdditiveAttention on 8 TRN2 NeuronCores — data-parallel over batch.

Algebraic restructuring: instead of materializing the [Lq,Lk,H] tanh
intermediate (33.5M elementwise ops/core), approximate

    tanh(z) ~= clin*z + alpha*sin(w*z)

(coefficients fit at runtime to the data's projection ranges; end-to-end
rel-err ~6e-3 vs the 2e-2 gate) and expand via the angle-sum identity

    sin(w(a+b)) = sin(wa)*(1-2*sin^2(wb/2)) + (1-2*sin^2(wa/2))*sin(wb)

so scores[q,k] = sum_h wv_h*tanh(qh+kh) collapse to a 3-row-per-h-chunk
matmul contraction (q-only terms drop out of the softmax):

    row 0: [wv*clin]_const(q)        x  kh_raw(k)        (linear term)
    row 1: [-2*a*wv*sin(w*qh)]       x  sin^2(w*kh/2)
    row 2: [a*wv*(1-2sin^2(w*qh/2))] x  sin(w*kh)

Features are sines of the small [Lq,H]/[Lk,H] projections; cos comes from
the half-angle square (respects Sin's [-pi,pi] hw range).  ACT evaluates
only Sin + final Exp (table load hidden behind a dummy exp); squares and
folds run on DVE; the kh->bf16 copy runs on idle GPSIMD.  Only
ceil(max_vlen/128) key slabs are processed; masking follows the
zeroed-values + mask-column trick (vlen==0 -> wv=0 -> uniform).
DMAs are bundled (one HWDGE generation each) and ordered by need since
the cost model serializes all DMA transfers on one resource.
"""

import ml_dtypes
import numpy as np

B, LQ, LK, D, H, DV = 8, 128, 1024, 512, 256, 512
NCORES = 8
HC = H // 128   # 2 h chunks
DC = D // 128   # 4 contraction chunks
NROW = 3        # contraction rows per h-chunk

# runtime-fit parameters (overwritten by _make_in_maps; affect numerics
# only, never the schedule)
_CFG = {"w": 1.30, "alph": 0.44, "clin": 0.35, "kce": 8}


def _build_program():
    import concourse.mybir as mybir
    import concourse.tile as tile
    from concourse import bacc

    f32 = mybir.dt.float32
    bf16 = mybir.dt.bfloat16
    AF = mybir.ActivationFunctionType
    mult = mybir.AluOpType.mult
    add = mybir.AluOpType.add
    w = _CFG["w"]
    KCe = _CFG["kce"]
    LKe = KCe * 128
    KW1 = LKe - 512
    NG2 = KCe - 4
    NCC = 8 + (KCe + 1) // 2  # f32 cols: wvm2a|wva|wkv|mcol(bf16-packed)

    nc = bacc.Bacc(
        "TRN2",
        target_bir_lowering=False,
        debug=False,
        num_devices=NCORES,
    )

    # bundled inputs: one HWDGE generation per DMA, ordered by need
    wkt0_ext = nc.dram_tensor("wkt0", [D, H + 256], bf16, kind="ExternalInput").ap()
    wqt_ext = nc.dram_tensor("wqt", [D, H + LQ], bf16, kind="ExternalInput").ap()
    ktq1_ext = nc.dram_tensor("ktq1", [D, 256], bf16, kind="ExternalInput").ap()
    kt1_ext = nc.dram_tensor("kt1", [D, KW1], bf16, kind="ExternalInput").ap()
    consts_ext = nc.dram_tensor("consts", [128, NCC], f32, kind="ExternalInput").ap()
    val_ext = nc.dram_tensor("values", [LKe, DV], bf16, kind="ExternalInput").ap()
    out_ext = nc.dram_tensor("out", [LQ, DV], bf16, kind="ExternalOutput").ap()

    with tile.TileContext(nc) as tc:
        with (
            tc.tile_pool(name="const", bufs=1) as const,
            tc.tile_pool(name="pq", bufs=1, space="PSUM") as pq,
            tc.tile_pool(name="pk", bufs=1, space="PSUM") as pk,
            tc.tile_pool(name="psc", bufs=1, space="PSUM") as psc,
            tc.tile_pool(name="pout", bufs=1, space="PSUM") as pout,
        ):
            # ---- SBUF residents ----------------------------------------
            wkt0 = const.tile([128, DC, H + 256], bf16, tag="wkt0")
            wqt = const.tile([128, DC, H + LQ], bf16, tag="wqt")
            ktq1 = const.tile([128, DC, 256], bf16, tag="ktq1")
            kt1 = const.tile([128, DC, KW1], bf16, tag="kt1")
            consts = const.tile([128, NCC], f32, tag="consts")
            vals = const.tile([128, KCe, DV], bf16, tag="vals")
            ones = const.tile([128, LQ], bf16, tag="ones")
            wkvq = const.tile([128, DC, LQ], bf16, tag="wkvq")
            asin = const.tile([128, HC, LQ], bf16, tag="asin")
            ahalf = const.tile([128, HC, LQ], bf16, tag="ahalf")
            ata = const.tile([128, HC, LQ], bf16, tag="ata")
            Arows = const.tile([128, HC, 2, LQ], bf16, tag="Arows")
            # per-quarter/half feature tiles (dep tracking is tile-granular)
            tbA = const.tile([128, HC, 256], bf16, tag="tbA")
            sbA = const.tile([128, HC, 256], bf16, tag="sbA")
            bhA = const.tile([128, HC, 256], bf16, tag="bhA")
            tbB = const.tile([128, HC, 256], bf16, tag="tbB")
            sbB = const.tile([128, HC, 256], bf16, tag="sbB")
            bhB = const.tile([128, HC, 256], bf16, tag="bhB")
            tb1t = const.tile([128, HC, KW1], bf16, tag="tb1t")
            sb1t = const.tile([128, HC, KW1], bf16, tag="sb1t")
            bhalf1 = const.tile([128, HC, KW1], bf16, tag="bhalf1")
            pT1 = const.tile([128, 4, LQ], bf16, tag="pT1")
            pT2 = const.tile([128, NG2, LQ], bf16, tag="pT2")
            rinv = const.tile([LQ, 1], f32, tag="rinv")
            out_a = const.tile([LQ, DV // 2], bf16, tag="outa")
            out_b = const.tile([LQ, DV // 2], bf16, tag="outb")

            wk_sb = wkt0[:, :, 0:H]
            ksA = wkt0[:, :, H:H + 256]      # kT columns 0:256
            wq_sb = wqt[:, :, 0:H]
            qsT = wqt[:, :, H:H + LQ]
            wvm2a = consts[:, 0:HC]
            wva = consts[:, HC:2 * HC]
            wkv = consts[:, 4:4 + DC]
            mcol = consts[:, 8:NCC].bitcast(bf16)

            nc.vector.memset(ones[:], 1.0)

            # ---- DMAs (transfers serialize globally in the cost model) -
            nc.sync.dma_start(
                wkt0[:], wkt0_ext.rearrange("(c p) x -> p c x", p=128)
            )
            nc.sync.dma_start(
                ktq1[:], ktq1_ext.rearrange("(c p) x -> p c x", p=128)
            )
            nc.sync.dma_start(
                wqt[:], wqt_ext.rearrange("(c p) x -> p c x", p=128)
            )
            nc.sync.dma_start(
                kt1[:], kt1_ext.rearrange("(c p) x -> p c x", p=128)
            )
            nc.sync.dma_start(consts[:], consts_ext[:])
            # values gated behind kt1's landing so it never delays the k-side
            nc.gpsimd.tensor_copy(vals[0:1, 0, 0:1], kt1[0:1, 0, 0:1])
            nc.gpsimd.dma_start(
                vals[:], val_ext.rearrange("(c p) v -> p c v", p=128)
            )

            # ---- PSUM tiles (8 banks exactly) --------------------------
            qh = pq.tile([128, HC, LQ], f32, tag="qh")
            khA = pk.tile([128, HC, 256], f32, tag="khA")
            khB = pk.tile([128, HC, 256], f32, tag="khB")
            kh1 = pk.tile([128, HC, KW1], f32, tag="kh1", padded_shape=[128, HC, 512])
            scg1 = psc.tile([128, 4, LQ], f32, tag="scg1")
            scg2 = psc.tile([128, NG2, LQ], f32, tag="scg2", padded_shape=[128, 4, LQ])
            po = pout.tile([LQ, DV], f32, tag="po")

            # ---- PE spins: hold the clock through every dep wait -------
            def spins(n, tgt):
                for _ in range(n):
                    nc.tensor.matmul(
                        tgt, lhsT=ones[:, 0:128], rhs=ones[:, 0:LQ],
                        start=True, stop=True, skip_group_check=True,
                    )

            def proj(dst, wsb, src, kw):
                for hc in range(HC):
                    for dc in range(DC):
                        nc.tensor.matmul(
                            dst[:, hc, 0:kw],
                            lhsT=wsb[:, dc, hc * 128:(hc + 1) * 128],
                            rhs=src[:, dc, 0:kw],
                            start=(dc == 0),
                            stop=(dc == DC - 1),
                        )

            # PE queue: spins / projA / projB / qh / proj1 / scores
            spins(30, scg1[:, 0, :])
            proj(khA, wk_sb, ksA, 256)
            spins(2, scg1[:, 0, :])
            proj(khB, wk_sb, ktq1, 256)
            spins(1, scg1[:, 0, :])
            proj(qh, wq_sb, qsT, LQ)
            spins(10, scg1[:, 0, :])
            proj(kh1, wk_sb, kt1, KW1)

            # ---- ACT stream (Sin only until the final Exp) -------------
            nc.scalar.activation(bhA[:], khA[:, :, :], AF.Sin, scale=w / 2)
            nc.scalar.activation(sbA[:], khA[:, :, :], AF.Sin, scale=w)
            nc.scalar.activation(bhB[:], khB[:, :, :], AF.Sin, scale=w / 2)
            nc.scalar.activation(asin[:, :, :], qh[:, :, :], AF.Sin, scale=w)
            nc.scalar.activation(ahalf[:, :, :], qh[:, :, :], AF.Sin, scale=w / 2)
            nc.scalar.activation(sbB[:], khB[:, :, :], AF.Sin, scale=w)
            nc.scalar.activation(bhalf1[:], kh1[:, :, 0:KW1], AF.Sin, scale=w / 2)
            nc.scalar.activation(sb1t[:], kh1[:, :, 0:KW1], AF.Sin, scale=w)

            # ---- DVE: wkv broadcast, squares, coefficient folds --------
            for dc in range(DC):
                nc.vector.tensor_scalar(
                    wkvq[:, dc, :], ones[:, 0:LQ], wkv[:, dc:dc + 1],
                    None, mult,
                )
            nc.vector.tensor_mul(tbA[:], bhA[:], bhA[:])
            nc.vector.tensor_mul(ata[:], ahalf[:], ahalf[:])
            for hc in range(HC):
                nc.vector.tensor_scalar(
                    Arows[:, hc, 0, :], asin[:, hc, :],
                    wvm2a[:, hc:hc + 1], None, mult,
                )
                nc.vector.tensor_scalar(
                    Arows[:, hc, 1, :], ata[:, hc, :],
                    wvm2a[:, hc:hc + 1], wva[:, hc:hc + 1], mult, add,
                )
            nc.vector.tensor_mul(tbB[:], bhB[:], bhB[:])
            nc.vector.tensor_mul(tb1t[:], bhalf1[:], bhalf1[:])

            # ---- scores: 8 accumulating matmuls per key slab -----------
            # linear row straight from the kT SBUF tiles (contraction d),
            # then the two sine rows (contraction h)
            def score_slabs(s0, s1, sc, g0, ktile, tbt, sbt):
                for s in range(s0, s1):
                    lf = s - s0   # index into feature/kT tiles
                    lg = s - g0   # index into the score-group tile
                    for dc in range(DC):
                        nc.tensor.matmul(
                            sc[:, lg, :],
                            lhsT=ktile[:, dc, lf * 128:(lf + 1) * 128],
                            rhs=wkvq[:, dc, :],
                            start=(dc == 0),
                            stop=False,
                        )
                    for r, rows in enumerate((tbt, sbt)):
                        for hc in range(HC):
                            nc.tensor.matmul(
                                sc[:, lg, :],
                                lhsT=rows[:, hc, lf * 128:(lf + 1) * 128],
                                rhs=Arows[:, hc, r, :],
                                start=False,
                                stop=(r == 1 and hc == HC - 1),
                            )

            score_slabs(0, 2, scg1, 0, ksA, tbA, sbA)
            spins(2, po[:, 0:LQ])
            score_slabs(2, 4, scg1, 0, ktq1, tbB, sbB)
            # last group: all linear rows first (ready early), then sines
            for s in range(4, KCe):
                lf = s - 4
                for dc in range(DC):
                    nc.tensor.matmul(
                        scg2[:, lf, :],
                        lhsT=kt1[:, dc, lf * 128:(lf + 1) * 128],
                        rhs=wkvq[:, dc, :],
                        start=(dc == 0), stop=False,
                    )
            spins(4, po[:, 0:LQ])
            for s in range(4, KCe):
                lf = s - 4
                for r, rows in enumerate((tb1t, sb1t)):
                    for hc in range(HC):
                        nc.tensor.matmul(
                            scg2[:, lf, :],
                            lhsT=rows[:, hc, lf * 128:(lf + 1) * 128],
                            rhs=Arows[:, hc, r, :],
                            start=False,
                            stop=(r == 1 and hc == HC - 1),
                        )
            spins(28, po[:, 0:LQ])

            # ---- softmax exp (first exp carries the table load) --------
            nc.scalar.activation(pT1[:], scg1[:], AF.Exp)
            nc.scalar.activation(pT2[:], scg2[:, 0:NG2, :], AF.Exp)

            ssum = pq.tile([LQ, 1], f32, tag="qh", name="ssum")
            for s in range(KCe):
                pt = pT1[:, s, :] if s < 4 else pT2[:, s - 4, :]
                nc.tensor.matmul(
                    ssum[:, :], lhsT=pt, rhs=mcol[:, s:s + 1],
                    start=(s == 0), stop=(s == KCe - 1),
                    skip_group_check=True,
                )
            nc.vector.reciprocal(rinv[:], ssum[:])
            # attn @ values in column halves; each half is normalized and
            # stored as soon as its accumulation completes
            for ch, outt in ((0, out_a), (1, out_b)):
                c0 = ch * (DV // 2)
                for s in range(KCe):
                    pt = pT1[:, s, :] if s < 4 else pT2[:, s - 4, :]
                    nc.tensor.matmul(
                        po[:, c0:c0 + DV // 2], lhsT=pt,
                        rhs=vals[:, s, c0:c0 + DV // 2],
                        start=(s == 0), stop=(s == KCe - 1),
                        skip_group_check=True,
                    )
                nc.vector.tensor_scalar_mul(
                    outt[:], po[:, c0:c0 + DV // 2], rinv[:]
                )
                nc.sync.dma_start(out_ext[:, c0:c0 + DV // 2], outt[:])

    nc.compile()
    return nc


def _fit_tanh(qh, kh):
    """Fit tanh(z) ~= clin*z + a*sin(w*z); w capped so every Sin argument
    (incl. half-angles) stays within [-pi, pi] on both sides."""
    amax = float(np.abs(qh).max())
    bmax = float(np.abs(kh).max())
    cmax = max(amax, bmax, 1e-3)
    sig = float(np.sqrt(qh.var() + kh.var()))
    sig = sig if sig > 1e-6 else 1.0
    wcap = np.pi / cmax / 1.01
    zmax = (amax + bmax) * 1.03
    zg = np.linspace(-zmax, zmax, 2001)
    wgt = np.exp(-0.5 * (zg / sig) ** 2) + 1e-3
    tz = np.tanh(zg)
    sww = np.sqrt(wgt)
    best = None
    for f1 in np.linspace(0.80, 0.995, 14):
        ws = wcap * f1
        A = np.stack([zg, np.sin(ws * zg)], axis=1)
        Aw = A * sww[:, None]
        G = Aw.T @ Aw + 1e-6 * np.eye(2)
        coef = np.linalg.solve(G, Aw.T @ (tz * sww))
        if np.abs(coef).sum() > 20:
            continue
        err = A @ coef - tz
        rms = float(np.sqrt((err ** 2 * wgt).sum() / wgt.sum()))
        mx = float(np.abs(err).max())
        s = rms + 0.01 * mx
        if best is None or s < best[0]:
            best = (s, ws, coef)
    _, ws, coef = best
    return float(ws), float(coef[1]), float(coef[0])


def _make_in_maps(queries, keys, values, Wq, Wk, wv, valid_lens):
    bfr = lambda x: np.asarray(x, np.float32).astype(ml_dtypes.bfloat16).astype(np.float32)
    queries = np.asarray(queries, dtype=np.float32)
    keys = np.asarray(keys, dtype=np.float32)
    values = np.asarray(values, dtype=np.float32)
    Wq = np.ascontiguousarray(np.asarray(Wq, dtype=np.float32))
    Wk = np.ascontiguousarray(np.asarray(Wk, dtype=np.float32))
    wv = np.asarray(wv, dtype=np.float32)
    vlens = np.asarray(valid_lens)

    qh = bfr(queries).reshape(-1, D) @ bfr(Wq)
    kh = bfr(keys).reshape(-1, D) @ bfr(Wk)
    w, alph, clin = _fit_tanh(qh, kh)
    _CFG["w"], _CFG["alph"], _CFG["clin"] = w, alph, clin
    if np.any(vlens == 0):
        KCe = 8
    else:
        KCe = max(1, int(-(-int(vlens.max()) // 128)))
    _CFG["kce"] = KCe
    LKe = KCe * 128

    Wq_bf = Wq.astype(ml_dtypes.bfloat16)
    Wk_bf = Wk.astype(ml_dtypes.bfloat16)
    wvT = np.ascontiguousarray(wv.reshape(HC, 128).T)  # [p, hc], h = hc*128+p
    karange = np.arange(LKe).reshape(KCe, 128).T  # [p, kc] -> k index
    in_maps = []
    for c in range(NCORES):
        vlen = int(vlens[c])
        if vlen == 0:
            mcol = np.ones((128, KCe), dtype=np.float32)
            wv_c = np.zeros_like(wvT)
            vals_c = values[c, :LKe]
        else:
            mcol = (karange < vlen).astype(np.float32)
            wv_c = wvT
            vals_c = np.where(
                (np.arange(LKe) < vlen)[:, None], values[c, :LKe], 0.0
            )
        mcol_bf = mcol.astype(ml_dtypes.bfloat16)
        if KCe % 2:
            mcol_bf = np.concatenate(
                [mcol_bf, np.zeros((128, 1), ml_dtypes.bfloat16)], axis=1
            )
        mcol_f32 = np.ascontiguousarray(mcol_bf).view(np.float32)
        wv_full = wv_c.T.reshape(-1)                       # [H], h = hc*128+p
        wkv = (clin * (Wk @ wv_full)).reshape(DC, 128).T    # [p, dc]
        consts = np.concatenate(
            [-2.0 * alph * wv_c, alph * wv_c, wkv, mcol_f32], axis=1
        ).astype(np.float32)
        kT = np.ascontiguousarray(keys[c].T).astype(ml_dtypes.bfloat16)
        in_maps.append(
            {
                "wkt0": np.ascontiguousarray(
                    np.concatenate([Wk_bf, kT[:, 0:256]], axis=1)
                ),
                "wqt": np.ascontiguousarray(
                    np.concatenate(
                        [Wq_bf, queries[c].T.astype(ml_dtypes.bfloat16)], axis=1
                    )
                ),
                "ktq1": np.ascontiguousarray(kT[:, 256:512]),
                "kt1": np.ascontiguousarray(kT[:, 512:LKe]),
                "consts": np.ascontiguousarray(consts),
                "values": np.ascontiguousarray(vals_c).astype(ml_dtypes.bfloat16),
            }
        )
    return in_maps


def kernel(queries, keys, values, Wq, Wk, wv, valid_lens):
    from concourse.bass_utils import run_bass_kernel_spmd

    in_maps = _make_in_maps(queries, keys, values, Wq, Wk, wv, valid_lens)
    nc = _build_program()
    res = run_bass_kernel_spmd(nc, in_maps, core_ids=list(range(NCORES)))
    out = np.stack(
        [res.results[c]["out"].astype(np.float32) for c in range(NCORES)], axis=0
    )
    return out
